# revision 37
# baseline (speedup 1.0000x reference)
"""Trainium2 Bass kernel for ChaoticAttentionLayer.

Math (reference):
    q = r_s * sig(zq) * (1 - sig(zq)),  zq = query @ Wq.T + bq,  r_s = 4*sigmoid(r)
    k likewise, v = value @ Wv.T + bv
    out = softmax(q k^T / 8) v @ Wo.T + bo   (per head, D=64)

Device decomposition:
    g = sig*(1-sig); scores = (r_s^2/8) * g(zq) . g(zk); the r_s^2/8 factor is
    folded into the Exp activation's scale. Scores are bounded in [0, 8] for
    any r, so softmax runs max-free: exp(scores) directly, denominator via an
    all-ones column appended to V.

Sharding: 8 cores = 4 batches x 2 head-groups (4 heads each). Each core
computes partial out[b] = attn_hg @ Wo[:, hg].T; host sums the two partials
per batch and adds bo.

Layout notes:
  - Everything is bf16 on the matmul paths (fp32r streams ~3.4x slower than
    modeled); projections accumulate in fp32 PSUM, softmax/normalization in
    fp32.
  - Scores are computed transposed, S^T[s_block, t], two heads per Exp call
    so the ACT instruction's 352-cycle fixed overhead is amortized over
    1024 columns.
  - attn V accumulators live 4-to-a-PSUM-bank; the first matmul into a bank
    uses start=True (bank-wide has_written clear), later t-sub groups'
    first matmuls use start=False and rely on per-element overwrite-when-
    unset semantics.
"""

import numpy as np
import ml_dtypes
from contextlib import ExitStack

try:
    import concourse.bass as bass
except ImportError:  # pragma: no cover
    import sys

    sys.path.insert(0, "/opt/trn_rl_repo")
    import concourse.bass as bass

import concourse.bacc as bacc
import concourse.tile as tile
from concourse import mybir
from concourse.bass_utils import run_bass_kernel_spmd
from concourse.masks import make_identity

F32 = mybir.dt.float32
BF16 = mybir.dt.bfloat16
AF = mybir.ActivationFunctionType
BF16NP = ml_dtypes.bfloat16

B, T, S, E, H = 4, 2048, 2048, 512, 8
D = E // H           # 64 head dim
HG = 2               # head-groups per batch (cores per batch)
HPG = H // HG        # 4 heads per group
EG = HPG * D         # 256 dims per head group
NCORES = 8
P = 128              # partitions
TCH = 512            # t-chunk (psum free dim)
NSB = S // P         # 16 s-blocks
NKT = E // P         # 4 contraction tiles of 128
NTC = T // TCH       # 4 t-chunks


def _build():
    nc = bacc.Bacc("TRN2", target_bir_lowering=False, debug=False,
                   num_devices=NCORES)

    xqT = nc.dram_tensor("xqT", [E, T], BF16, kind="ExternalInput")
    xkT = nc.dram_tensor("xkT", [E, S], BF16, kind="ExternalInput")
    xvT = nc.dram_tensor("xvT", [E + 1, S], BF16, kind="ExternalInput")
    wqT = nc.dram_tensor("wqT", [E, EG], BF16, kind="ExternalInput")
    wkT = nc.dram_tensor("wkT", [E, EG], BF16, kind="ExternalInput")
    wvT = nc.dram_tensor("wvT", [E + 1, EG], BF16, kind="ExternalInput")
    woT = nc.dram_tensor("woT", [EG, E], BF16, kind="ExternalInput")
    bq = nc.dram_tensor("bq", [EG, 1], F32, kind="ExternalInput")
    bk = nc.dram_tensor("bk", [EG, 1], F32, kind="ExternalInput")
    cexp = nc.dram_tensor("cexp", [1, 1], F32, kind="ExternalInput")
    out = nc.dram_tensor("out", [T, E], F32, kind="ExternalOutput")

    with tile.TileContext(nc) as tc, ExitStack() as ctx:
        persist = ctx.enter_context(tc.tile_pool(name="persist", bufs=1))

        # --- persistent SBUF state ---
        wq_sb = []
        wk_sb = []
        wv_sb = []
        for kt in range(NKT):
            tq = persist.tile([P, EG], BF16, tag=f"wq{kt}")
            nc.sync.dma_start(out=tq, in_=wqT[kt * P:(kt + 1) * P, :])
            wq_sb.append(tq)
            tk = persist.tile([P, EG], BF16, tag=f"wk{kt}")
            nc.sync.dma_start(out=tk, in_=wkT[kt * P:(kt + 1) * P, :])
            wk_sb.append(tk)
            tv = persist.tile([P, EG], BF16, tag=f"wv{kt}")
            nc.sync.dma_start(out=tv, in_=wvT[kt * P:(kt + 1) * P, :])
            wv_sb.append(tv)
        wv4_sb = persist.tile([1, EG], BF16, tag="wv4")
        nc.sync.dma_start(out=wv4_sb, in_=wvT[E:E + 1, :])

        wo_sb = []
        for kb in range(EG // P):
            to = persist.tile([P, E], BF16, tag=f"wo{kb}")
            nc.sync.dma_start(out=to, in_=woT[kb * P:(kb + 1) * P, :])
            wo_sb.append(to)

        bq_sb = []
        bk_sb = []
        for c in range(EG // P):
            tb_ = persist.tile([P, 1], F32, tag=f"bq{c}")
            nc.sync.dma_start(out=tb_, in_=bq[c * P:(c + 1) * P, :])
            bq_sb.append(tb_)
            tb2 = persist.tile([P, 1], F32, tag=f"bk{c}")
            nc.sync.dma_start(out=tb2, in_=bk[c * P:(c + 1) * P, :])
            bk_sb.append(tb2)

        cexp_sb = persist.tile([P, 1], F32, tag="cexp")
        cap = cexp[:, :]
        nc.sync.dma_start(
            out=cexp_sb,
            in_=bass.AP(tensor=cap.tensor, offset=cap.offset, ap=[[0, P], [1, 1]]),
        )

        ones_sb = persist.tile([1, S], BF16, tag="ones")
        nc.sync.dma_start(out=ones_sb, in_=xvT[E:E + 1, :])

        ident = persist.tile([P, P], BF16, tag="ident")
        make_identity(nc, ident)

        # projected tensors, resident for the whole kernel; chunked into
        # [P, TCH] column tiles so consumers unblock per-chunk.
        QT_sb = [[persist.tile([P, TCH], BF16, tag=f"qt{c}_{q}",
                               name=f"qt{c}_{q}") for q in range(NTC)]
                 for c in range(EG // P)]
        KT_sb = [[persist.tile([P, TCH], BF16, tag=f"kt{c}_{q}",
                               name=f"ktile{c}_{q}") for q in range(NTC)]
                 for c in range(EG // P)]
        V_sb = [persist.tile([P, HPG, D + 1], BF16, tag=f"v{sc}", name=f"v{sc}")
                for sc in range(NSB)]

        # --- projections (x inputs resident in SBUF as big tiles) ---
        with ExitStack() as c2:
            xk_sb = [[None] * NTC for _ in range(NKT)]
            xv_sb = [[None] * NTC for _ in range(NKT)]
            xq_sb = [[None] * NTC for _ in range(NKT)]
            for q in range(NTC):
                for kt in range(NKT):
                    xk_t = persist.tile([P, TCH], BF16, tag=f"xk{kt}_{q}",
                                        name=f"xk{kt}_{q}")
                    nc.sync.dma_start(
                        out=xk_t,
                        in_=xkT[kt * P:(kt + 1) * P, q * TCH:(q + 1) * TCH])
                    xk_sb[kt][q] = xk_t
                for kt in range(NKT):
                    xq_t = persist.tile([P, TCH], BF16, tag=f"xq{kt}_{q}",
                                        name=f"xq{kt}_{q}")
                    nc.sync.dma_start(
                        out=xq_t,
                        in_=xqT[kt * P:(kt + 1) * P, q * TCH:(q + 1) * TCH])
                    xq_sb[kt][q] = xq_t
                for kt in range(NKT):
                    xv_t = persist.tile([P, TCH], BF16, tag=f"xvr{kt}_{q}",
                                        name=f"xvr{kt}_{q}")
                    nc.sync.dma_start(
                        out=xv_t,
                        in_=xvT[kt * P:(kt + 1) * P, q * TCH:(q + 1) * TCH])
                    xv_sb[kt][q] = xv_t



            def qk_proj_chunk(pool, x_sb, w_sb, b_sb, out_tiles, c, tcq,
                              sig):
                ps = pool.tile([P, TCH], F32, tag="ps", name=f"ps_{c}_{tcq}")
                for kt in range(NKT):
                    nc.tensor.matmul(
                        ps, w_sb[kt][:, c * P:(c + 1) * P],
                        x_sb[kt][tcq],
                        start=(kt == 0), stop=(kt == NKT - 1))
                # sig'(z) = (1 - tanh^2(z/2)) / 4 -- tanh shares ACT's exp
                # table set, so the whole kernel uses one ACT_TABLE_LOAD.
                y = sig.tile([P, TCH], F32, tag="y", name=f"y_{c}_{tcq}")
                nc.scalar.activation(y, ps, AF.Tanh, bias=b_sb[c], scale=0.5)
                y2 = sig.tile([P, TCH], F32, tag="y2", name=f"y2_{c}_{tcq}")
                nc.vector.tensor_mul(y2, y, y)
                nc.vector.tensor_scalar(out_tiles[c][tcq], y2, -0.25, 0.25,
                                        mybir.AluOpType.mult,
                                        mybir.AluOpType.add)

            def v_proj_unit(tcq, sci, psp):
                sc = tcq * 4 + sci
                ps = psp.tile([P, TCH], F32, tag="ps", name=f"psv_{sc}")
                for kt in range(NKT):
                    nc.tensor.matmul(ps[:, 0:EG],
                                     xv_sb[kt][tcq][:, sci * P:(sci + 1) * P],
                                     wv_sb[kt], start=(kt == 0), stop=False)
                nc.tensor.matmul(ps[:, 0:EG], ones_sb[:, sc * P:(sc + 1) * P],
                                 wv4_sb, start=False, stop=True)
                nc.vector.tensor_copy(
                    V_sb[sc][:, :, 0:D],
                    ps[:, 0:EG].rearrange("p (h d) -> p h d", h=HPG))
                nc.vector.memset(V_sb[sc][:, :, D:D + 1], 1.0)

            def proj_units(tcq, psp, sig):
                units = []
                for c in range(EG // P):
                    units.append(lambda c=c: qk_proj_chunk(
                        psp, xk_sb, wk_sb, bk_sb, KT_sb, c, tcq, sig))
                for c in range(EG // P):
                    units.append(lambda c=c: qk_proj_chunk(
                        psp, xq_sb, wq_sb, bq_sb, QT_sb, c, tcq, sig))
                for sci in range(4):
                    units.append(lambda sci=sci: v_proj_unit(tcq, sci, psp))
                return units

            def proj_group(tcq, psp, sig):
                for u in proj_units(tcq, psp, sig):
                    u()

        # --- attention + out-projection (interleaved with Q proj) ---
        # Heads processed in pairs: one Exp call covers both heads' scores
        # (1024 columns) to amortize ACT fixed overhead. Q projection for
        # t-chunk i is emitted right before attention on t-chunk i so the
        # attention pipeline (and ACT) starts as soon as K/V are done; it
        # borrows scores-pool tiles (uses the first 512 cols) to stay inside
        # the 8-bank PSUM budget.
            c3 = c2
            expp = c3.enter_context(tc.tile_pool(name="expp", bufs=6))
            attnp = c3.enter_context(tc.tile_pool(name="attnp", bufs=3))
            atp = c3.enter_context(tc.tile_pool(name="atp", bufs=2))
            rdp = c3.enter_context(tc.tile_pool(name="rdp", bufs=8))
            outp = c3.enter_context(tc.tile_pool(name="outp", bufs=3))

            def attention_tci(tci, pss, psav, pst, psf, fillers=()):
                attn_ts = [attnp.tile([P, EG], BF16, tag=f"ao{ts}",
                                      name=f"ao_{tci}_{ts}")
                           for ts in range(4)]
                for hp in range(HPG // 2):        # head pairs
                    # two av accumulator banks, 4 t-subs each at 65-col pitch
                    avs = [psav.tile([P, 4 * (D + 1)], F32, tag=f"av{i}",
                                     name=f"av_{tci}_{hp}_{i}")
                           for i in range(2)]
                    for sb in range(NSB):
                        ps = pss.tile([P, 2 * TCH], F32, tag="ps",
                                      name=f"ps_{tci}_{hp}_{sb}")
                        for hi in range(2):
                            h = 2 * hp + hi
                            ch, off = h // 2, (h % 2) * D
                            nc.tensor.matmul(
                                ps[:, hi * TCH:(hi + 1) * TCH],
                                KT_sb[ch][sb // 4][off:off + D,
                                                   (sb % 4) * P:
                                                   (sb % 4 + 1) * P],
                                QT_sb[ch][tci][off:off + D, :],
                                start=True, stop=True,
                                tile_position=(off, 0))
                        ex = expp.tile([P, 2 * TCH], BF16, tag="ex")
                        nc.scalar.activation(ex, ps, AF.Exp, scale=cexp_sb)
                        for hi in range(2):
                            for ts in range(4):
                                nc.tensor.matmul(
                                    avs[hi][:, ts * (D + 1):(ts + 1) * (D + 1)],
                                    ex[:, hi * TCH + ts * P:
                                       hi * TCH + (ts + 1) * P],
                                    V_sb[sb][:, 2 * hp + hi, :],
                                    start=(sb == 0 and ts == 0),
                                    stop=(sb == NSB - 1),
                                    skip_group_check=not (sb == 0 and ts == 0))
                    for hi in range(2):
                        h = 2 * hp + hi
                        for ts in range(4):
                            col = ts * (D + 1)
                            rd = rdp.tile([P, 1], F32, tag="rd")
                            nc.vector.reciprocal(
                                rd, avs[hi][:, col + D:col + D + 1])
                            nc.vector.tensor_scalar_mul(
                                attn_ts[ts][:, h * D:(h + 1) * D],
                                avs[hi][:, col:col + D], rd)
                    if hp == 0 and fillers:
                        for u in fillers:
                            u()


                aT = [atp.tile([P, TCH], BF16, tag=f"at{kb}",
                               name=f"at_{tci}_{kb}")
                      for kb in range(EG // P)]
                for ts in range(4):
                    for kb in range(EG // P):
                        pt = pst.tile([P, P], BF16, tag="av0",
                                      name=f"pt_{tci}_{ts}_{kb}")
                        nc.tensor.transpose(
                            pt, attn_ts[ts][:, kb * P:(kb + 1) * P], ident)
                        nc.vector.tensor_copy(aT[kb][:, ts * P:(ts + 1) * P], pt)
                for ts in range(4):
                    pf = psf.tile([P, E], F32, tag="av1", name=f"pf_{tci}_{ts}")
                    for kb in range(EG // P):
                        nc.tensor.matmul(pf, aT[kb][:, ts * P:(ts + 1) * P],
                                         wo_sb[kb], start=(kb == 0),
                                         stop=(kb == EG // P - 1))
                    ot = outp.tile([P, E], F32, tag="ot")
                    nc.vector.tensor_copy(ot, pf)
                    row = (tci * 4 + ts) * P
                    nc.sync.dma_start(out=out[row:row + P, :], in_=ot)

            psp = c2.enter_context(tc.tile_pool(name="psp", bufs=2, space="PSUM"))
            sig = c2.enter_context(tc.tile_pool(name="sig", bufs=4))
            pss = c2.enter_context(tc.tile_pool(name="pss", bufs=2, space="PSUM"))
            psav = c2.enter_context(tc.tile_pool(name="psav", bufs=1, space="PSUM"))
            # K and V are consumed across ALL s-blocks by every t-chunk, so
            # they must be fully projected before attention starts. Only the
            # Q chunks are t-chunk-local and pipeline into the attention loop.
            for tcq in range(NTC):
                for c in range(EG // P):
                    qk_proj_chunk(psp, xk_sb, wk_sb, bk_sb, KT_sb, c, tcq, sig)
            for tcq in range(NTC):
                for sci in range(4):
                    v_proj_unit(tcq, sci, psp)
            for c in range(EG // P):
                qk_proj_chunk(psp, xq_sb, wq_sb, bq_sb, QT_sb, c, 0, sig)
            for tcq in range(NTC):
                if tcq + 1 < NTC:
                    nxt = tuple(
                        (lambda c=c, q=tcq + 1: qk_proj_chunk(
                            psp, xq_sb, wq_sb, bq_sb, QT_sb, c, q, sig))
                        for c in range(EG // P))
                else:
                    nxt = ()
                attention_tci(tcq, pss, psav, psav, psav, fillers=nxt)

    nc.compile()
    return nc


_NC = None
_LAST_IN_MAPS = None


def _get_nc():
    global _NC
    if _NC is None:
        _NC = _build()
    return _NC


def kernel(**inputs):
    query = np.asarray(inputs["query"], np.float32)
    key_ = np.asarray(inputs["key_"] if "key_" in inputs else inputs["key"],
                      np.float32)
    value = np.asarray(inputs["value"], np.float32)
    Wq = np.asarray(inputs["Wq"], np.float32)
    bq = np.asarray(inputs["bq"], np.float32)
    Wk = np.asarray(inputs["Wk"], np.float32)
    bk = np.asarray(inputs["bk"], np.float32)
    Wv = np.asarray(inputs["Wv"], np.float32)
    bv = np.asarray(inputs["bv"], np.float32)
    Wo = np.asarray(inputs["Wo"], np.float32)
    bo = np.asarray(inputs["bo"], np.float32)
    r = float(np.asarray(inputs["r"]).reshape(-1)[0])

    r_s = 4.0 / (1.0 + np.exp(-np.float64(r)))
    c = np.float32(r_s * r_s / 8.0)

    WqT = Wq.T.astype(BF16NP)
    WkT = Wk.T.astype(BF16NP)
    WoT = Wo.T.astype(BF16NP)
    WvTa = np.concatenate([Wv.T, bv[None, :]], axis=0).astype(BF16NP)

    in_maps = []
    for b in range(B):
        xqT = np.ascontiguousarray(query[b].T).astype(BF16NP)
        xkT = np.ascontiguousarray(key_[b].T).astype(BF16NP)
        xvT = np.concatenate(
            [np.ascontiguousarray(value[b].T), np.ones((1, S), np.float32)],
            axis=0).astype(BF16NP)
        for g in range(HG):
            cols = slice(g * EG, (g + 1) * EG)
            in_maps.append(dict(
                xqT=xqT, xkT=xkT, xvT=xvT,
                wqT=np.ascontiguousarray(WqT[:, cols]),
                wkT=np.ascontiguousarray(WkT[:, cols]),
                wvT=np.ascontiguousarray(WvTa[:, cols]),
                woT=np.ascontiguousarray(WoT[cols, :]),
                bq=np.ascontiguousarray(0.5 * bq[cols, None]),
                bk=np.ascontiguousarray(0.5 * bk[cols, None]),
                cexp=np.array([[c]], np.float32),
            ))

    global _LAST_IN_MAPS
    _LAST_IN_MAPS = in_maps
    res = run_bass_kernel_spmd(_get_nc(), in_maps, core_ids=list(range(NCORES)))
    out = np.empty((B, T, E), np.float32)
    for b in range(B):
        out[b] = res.results[HG * b]["out"]
        for g in range(1, HG):
            out[b] += res.results[HG * b + g]["out"]
        out[b] += bo[None, :]
    return out
